# revision 20
# baseline (speedup 1.0000x reference)
"""Trainium2 Bass kernel for the 81-step LSTM decoder + masked softmax.

Math (per batch row b):
    z_t = x_t @ W_x + h_{t-1} @ W_h + b          (gates i, f, g, o; 100 each)
    i,f,o = sigmoid;  g = identity
    c_t = f*c_{t-1} + i*g;  h_t = o*c_t
    out_t = softmax(where(mask_t, h_t, -inf))

Strategy: data-parallel over batch (4096 -> 8 cores x 512), h-major on device
(hidden dim on partitions, batch on free dim), identical program per core.

The x-projection runs in fp8-e4m3 DoubleRow mode (0.5 PE cycles/row, two
128-row K-tiles per instruction).  Everything is scaled by S=64 on device
(W8 = q8(S*W_x) keeps the fp8 residual of W representable) and un-scaled for
free inside the activations via ACT's scale argument:
  i,f,o gates: 1-term  z ~ x8 @ W8            (sigmoid damps fp8 noise)
  g gate:      3-term  x8@W8 + xlo@W8 + x8@Wlo (g feeds c linearly -> bf16-
               quality needed; pairs ride in the DoubleRow k-tile slots)
The recurrent matmul stays bf16 (1 cyc/row).  The LSTM elementwise chain runs
in bf16 on DVE (2x packed mode), split into two independent batch halves so
the two recurrence chains pipeline across engines.  The masked softmax:
exp(h/S) on ACT (3-step chunks), then a "masked transpose" per step -- a
plain matmul  e^T @ diag(mask01_t)  that transposes to batch-major and
applies the softmax mask in one PE pass -- then sum + normalize on the
(otherwise idle) GPSIMD engine, reciprocal on DVE.  Output is written bf16
and upconverted to fp32 on the host.
"""

import sys

if "/opt/trn_rl_repo" not in sys.path:
    sys.path.insert(0, "/opt/trn_rl_repo")

import numpy as np

P = 81       # places / timesteps
H = 100      # LSTM units
E = 512      # encoder feature width
B = 4096     # total batch
NCORES = 8
BS = B // NCORES          # 512 batch rows per core
NB = BS // 128            # 4 batch tiles of 128
NE = E // 128             # 4 feature chunks of 128
HS = BS // 2              # 256: batch half for the split recurrence chains
K = 9                     # softmax window (81 % 9 == 0)
S = 64.0                  # global gate scale (keeps fp8 W residual normal)

# gate blocks in device column order: 0=f 1=i 2=o 3=g
# reference W column order is i,f,g,o -> reorder slices
_REORD = [slice(100, 200), slice(0, 100), slice(300, 400), slice(200, 300)]

_PROGRAM = None


def _build_program():
    import concourse.bacc as bacc
    import concourse.mybir as mybir
    from concourse.tile import TileContext
    from concourse.tile_rust import add_dep_helper
    from contextlib import ExitStack

    f32 = mybir.dt.float32
    bf16 = mybir.dt.bfloat16
    fp8 = mybir.dt.float8e4
    SIG = mybir.ActivationFunctionType.Sigmoid
    EXP = mybir.ActivationFunctionType.Exp
    DR = mybir.MatmulPerfMode.DoubleRow
    ADD = mybir.AluOpType.add
    X = mybir.AxisListType.X

    nc = bacc.Bacc(None, target_bir_lowering=False)

    xin_d = nc.dram_tensor("xin", [P, 128, 4096], fp8, kind="ExternalInput")
    w8p_d = nc.dram_tensor("w8p", [128, 2, 4, 2, 112], fp8, kind="ExternalInput")
    wlp_d = nc.dram_tensor("wlp", [128, 2, 2, 112], fp8, kind="ExternalInput")
    whb_d = nc.dram_tensor("whb", [H + 1, 400], bf16, kind="ExternalInput")
    dmk_d = nc.dram_tensor("dmk", [H, P * (H + 1)], bf16, kind="ExternalInput")
    h0T_d = nc.dram_tensor("h0T", [H + 1, BS], bf16, kind="ExternalInput")
    out_d = nc.dram_tensor("out", [BS, P, H], bf16, kind="ExternalOutput")

    with ExitStack() as ctx:
        tc = ctx.enter_context(TileContext(nc))
        consts = ctx.enter_context(tc.tile_pool(name="consts", bufs=1))
        xpool = ctx.enter_context(tc.tile_pool(name="xpool", bufs=12))
        gpool = ctx.enter_context(tc.tile_pool(name="gpool", bufs=8))
        opool = ctx.enter_context(tc.tile_pool(name="opool", bufs=8))
        zpool = ctx.enter_context(tc.tile_pool(name="zpool", bufs=2, space="PSUM"))
        zgpool = ctx.enter_context(tc.tile_pool(name="zgpool", bufs=1, space="PSUM"))
        epool = ctx.enter_context(tc.tile_pool(name="epool", bufs=1, space="PSUM"))

        # prefetch the first x tiles ahead of the big consts DMAs so the PE
        # can start immediately
        xtiles = {}
        for t0 in range(4):
            xtiles[t0] = xpool.tile([128, 4096], fp8, name=f"x_{t0}", tag="x")
            nc.sync.dma_start(out=xtiles[t0], in_=xin_d[t0])

        w8p = consts.tile([128, 2, 4, 2, 112], fp8)
        nc.sync.dma_start(out=w8p, in_=w8p_d[:, :, :, :, :])
        wlp = consts.tile([128, 2, 2, 112], fp8)
        nc.sync.dma_start(out=wlp, in_=wlp_d[:, :, :, :])
        whb = consts.tile([H + 1, 400], bf16)
        nc.sync.dma_start(out=whb, in_=whb_d[:, :])
        dmk = consts.tile([H, P, H + 1], bf16)
        nc.sync.dma_start(
            out=dmk, in_=dmk_d[:, :].rearrange("p (t h) -> p t h", h=H + 1)
        )

        # h history ring: rows 0:100 = S*h_t (bf16), row 100 = 1.0 (bias rider)
        hist = consts.tile([H + 1, K, BS], bf16)
        nc.sync.dma_start(out=hist[:, K - 1, :], in_=h0T_d[:, :])
        for j in range(K - 1):
            nc.sync.dma_start(out=hist[H : H + 1, j, :], in_=h0T_d[H : H + 1, :])
        cC = consts.tile([H, BS], bf16)       # S*c_t, persistent
        nc.vector.memset(cC, 0.0)
        ew = consts.tile([H, K, BS], bf16)    # exp(h/S) window staging

        # ACT LUT ops chained in program order (avoid sigmoid<->exp table churn)
        act_prev = [None]

        def act_ordered(bi):
            if act_prev[0] is not None:
                add_dep_helper(bi.ins, act_prev[0].ins, sync=False, reason="act order")
            act_prev[0] = bi

        def softmax_tail(tau):
            slot = tau % K
            # masked transpose + fused masked sum:
            #   eT[b, h] = sum_hh e[hh, b] * D[hh, h]   (D = diag(mask01_t))
            #   eT[b, 100] = sum_hh e[hh, b] * mask01_t[hh]   (softmax denom)
            eT = epool.tile([128, NB, H + 1], f32, name=f"eT_{tau}", tag="eT")
            for k in range(NB):
                nc.tensor.matmul(
                    eT[:, k, :],
                    ew[:, slot, 128 * k : 128 * (k + 1)],
                    dmk[:, tau, :],
                    start=True,
                    stop=True,
                )
            r = opool.tile([128, NB], f32, name=f"r_{tau}", tag="r")
            nc.vector.reciprocal(r, eT[:, :, H])
            ostg = opool.tile([128, NB, H], bf16, name=f"o_{tau}", tag="ostg")
            nc.vector.tensor_mul(
                ostg, eT[:, :, 0:H], r[:, :, None].broadcast_to([128, NB, H])
            )
            nc.sync.dma_start(
                out=out_d.rearrange("(k p) t h -> p k t h", p=128)[:, :, tau, :],
                in_=ostg,
            )

        due = {}

        def emit_due(t):
            # deferred exp chunks read hist slots that this step's h-write is
            # about to overwrite, so they must precede the step body
            for kind2, arg in due.pop(t, []):
                if kind2 == "exp":
                    act_ordered(
                        nc.scalar.activation(
                            ew[:, arg : arg + 3, :],
                            hist[0:H, arg : arg + 3, :],
                            EXP,
                            scale=1.0 / S,
                        )
                    )
                else:
                    softmax_tail(arg)

        for t in range(P):
            emit_due(t)
            if t in xtiles:
                xt = xtiles[t]
            else:
                xt = xpool.tile([128, 4096], fp8, name=f"x_{t}", tag="x")
                nc.sync.dma_start(out=xt, in_=xin_d[t])

            # contiguous [2, 512] DR moving pair at (kind, ep)
            def xpair(kind, ep):
                off = kind * 2048 + ep * 1024
                return xt[:, off : off + 1024].rearrange("p (j n) -> p j n", j=2)

            prev = (t - 1) % K
            zf = zpool.tile([H, 3, BS], f32, name=f"zf_{t}", tag="zf")
            zg = zgpool.tile([H, BS], f32, name=f"zg_{t}", tag="zg")

            # ---- x-projection (no h dependency) ----
            # DoubleRow start=True zeroes the whole PSUM bank, so exactly one
            # start per bank per step: the bank's first matmul. All regions
            # accumulate per-address after that (skip_group_check).
            for blk in range(3):               # f, i, o : 1-term fp8 DR
                for ep in range(2):
                    nc.tensor.matmul(
                        zf[:, blk, :],
                        w8p[:, ep, blk, :, 0:100],
                        xpair(0, ep),
                        start=(ep == 0),
                        stop=False,
                        perf_mode=DR,
                        skip_group_check=True,
                    )
            # g : 3-term fp8 DR (x8@W8g + xlo@W8g + x8@Wlog)
            for ti, (kind, wt) in enumerate(((0, w8p[:, :, 3]), (1, w8p[:, :, 3]), (0, wlp))):
                for ep in range(2):
                    nc.tensor.matmul(
                        zg[:, :],
                        wt[:, ep, :, 0:100],
                        xpair(kind, ep),
                        start=(ti == 0 and ep == 0),
                        stop=False,
                        perf_mode=DR,
                        skip_group_check=True,
                    )

            # ---- recurrent part + gate chain, per batch half ----
            for blk in range(3):
                nc.tensor.matmul(
                    zf[:, blk, :],
                    whb[:, 100 * blk : 100 * (blk + 1)],
                    hist[:, prev, :],
                    start=False,
                    stop=True,
                    skip_group_check=True,
                )
            nc.tensor.matmul(
                zg[:, :],
                whb[:, 300:400],
                hist[:, prev, :],
                start=False,
                stop=True,
                skip_group_check=True,
            )

            for hf in range(2):
                bs_ = slice(HS * hf, HS * (hf + 1))
                fio = gpool.tile([H, 3, BS], bf16, name=f"fio_{t}_{hf}", tag="fio")
                act_ordered(
                    nc.scalar.activation(
                        fio[:, :, bs_], zf[:, :, bs_], SIG, scale=1.0 / S
                    )
                )
                t1 = gpool.tile([H, BS], bf16, name=f"t1_{t}_{hf}", tag="t1")
                nc.gpsimd.tensor_mul(t1[:, bs_], fio[:, 0, bs_], cC[:, bs_])
                t2 = gpool.tile([H, BS], bf16, name=f"t2_{t}_{hf}", tag="t2")
                nc.vector.tensor_mul(t2[:, bs_], fio[:, 1, bs_], zg[:, bs_])
                nc.vector.tensor_add(cC[:, bs_], t1[:, bs_], t2[:, bs_])
                nc.vector.tensor_mul(
                    hist[0:H, t % K, bs_], fio[:, 2, bs_], cC[:, bs_]
                )

            if t % K == K - 1:
                w0 = t - K + 1
                if t + 1 < P:
                    sched = [
                        (t + 1, [("exp", 0), ("tail", w0)]),
                        (t + 2, [("tail", w0 + 1), ("tail", w0 + 2)]),
                        (t + 3, [("exp", 3), ("tail", w0 + 3)]),
                        (t + 4, [("tail", w0 + 4), ("tail", w0 + 5)]),
                        (t + 5, [("exp", 6), ("tail", w0 + 6)]),
                        (t + 6, [("tail", w0 + 7), ("tail", w0 + 8)]),
                    ]
                    for st, evs in sched:
                        due.setdefault(st, []).extend(evs)
                else:
                    for c0 in range(0, K, 3):
                        act_ordered(
                            nc.scalar.activation(
                                ew[:, c0 : c0 + 3, :],
                                hist[0:H, c0 : c0 + 3, :],
                                EXP,
                                scale=1.0 / S,
                            )
                        )
                    for tau in range(w0, t + 1):
                        softmax_tail(tau)

    nc.compile()
    return nc


def _get_program():
    global _PROGRAM
    if _PROGRAM is None:
        _PROGRAM = _build_program()
    return _PROGRAM


def _prep_in_maps(h_enc, h0, W_x, W_h, b, mask):
    import ml_dtypes

    e4 = ml_dtypes.float8_e4m3
    bf16 = ml_dtypes.bfloat16
    h_enc = np.asarray(h_enc, dtype=np.float32)
    h0 = np.asarray(h0, dtype=np.float32)
    W_x = np.asarray(W_x, dtype=np.float32)
    W_h = np.asarray(W_h, dtype=np.float32)
    b = np.asarray(b, dtype=np.float32)
    mask = np.asarray(mask)

    def q8(x):
        return np.clip(x, -240.0, 240.0).astype(e4)

    # reorder W columns to (f, i, o, g)
    Wp = np.concatenate([W_x[:, s] for s in _REORD], axis=1)          # [512, 400]
    bp = np.concatenate([b[s] for s in _REORD])                       # [400]
    Whp = np.concatenate([W_h[:, s] for s in _REORD], axis=1)         # [100, 400]

    W8 = q8(S * Wp)
    Wlo = q8(S * Wp - W8.astype(np.float32))
    W8c = W8.reshape(NE, 128, 400)
    Wloc = Wlo.reshape(NE, 128, 400)
    # W8 pairs for all 4 gate blocks: [128, ep, blk, j, 112], (ep,j) -> e-chunk
    # 2*ep+j (pair stride padded to 112: DoubleRow ldweights needs stride%16==0)
    w8p = np.zeros((128, 2, 4, 2, 112), e4)
    w8p[:, :, :, :, 0:100] = (
        W8c.reshape(2, 2, 128, 4, 100)     # [ep, j, p, blk, 100]
        .transpose(2, 0, 3, 1, 4)          # [p, ep, blk, j, 100]
    )
    # g term3: Wlo pairs: [128, ep, j, 112]
    wlp = np.zeros((128, 2, 2, 112), e4)
    wlp[:, :, :, 0:100] = (
        Wloc[:, :, 300:400].reshape(2, 2, 128, 100).transpose(2, 0, 1, 3)
    )

    whb = np.concatenate([Whp, (S * bp)[None, :]], axis=0).astype(bf16)  # [101, 400]

    # masked-transpose blocks: dmk[hh, t, h<100] = (hh==h)*mask01[t,h];
    # column 100 = mask01[t, hh] (produces the masked softmax sum in-pass)
    m01 = mask.astype(np.float32)                                     # [81, 100]
    dmk = np.zeros((H, P, H + 1), np.float32)
    ii = np.arange(H)
    dmk[ii, :, ii] = m01.T[ii]                                        # diag per t
    dmk[:, :, H] = m01.T                                              # sum column
    dmk = dmk.reshape(H, P * (H + 1)).astype(bf16)

    in_maps = []
    xT = np.empty((P, E, BS), np.float32)
    for c in range(NCORES):
        shard = h_enc[c * BS : (c + 1) * BS]                          # [BS, P, E]
        for t in range(P):
            xT[t] = shard[:, t, :].T
        x8 = q8(xT)
        xlo = q8(xT - x8.astype(np.float32))
        # xin [P, 128, kind, ep, j, 512]: each DR rhs a contiguous [2,512]
        xin = np.empty((P, 128, 2, 2, 2, 512), e4)
        for kind, xq in ((0, x8), (1, xlo)):
            a = xq.reshape(P, 2, 2, 128, 512)           # [t, ep, j, p, n]
            xin[:, :, kind] = a.transpose(0, 3, 1, 2, 4)
        xin = xin.reshape(P, 128, 4096)
        h0T = np.concatenate(
            [S * h0[c * BS : (c + 1) * BS].T, np.ones((1, BS), np.float32)], axis=0
        ).astype(bf16)
        in_maps.append(
            {
                "xin": xin,
                "w8p": w8p,
                "wlp": wlp,
                "whb": whb,
                "dmk": dmk,
                "h0T": np.ascontiguousarray(h0T),
            }
        )
    return in_maps


def run(inputs: dict, trace: bool = False):
    """Run on 8 cores; returns (full_output, exec_time_ns_or_None)."""
    from concourse.bass_utils import run_bass_kernel_spmd

    nc = _get_program()
    in_maps = _prep_in_maps(**inputs)
    res = run_bass_kernel_spmd(
        nc, in_maps, core_ids=list(range(NCORES)), trace=trace
    )
    out = np.concatenate(
        [r["out"].astype(np.float32) for r in res.results], axis=0
    )
    return out, res.exec_time_ns


def kernel(**inputs) -> np.ndarray:
    out, _ = run(inputs, trace=False)
    return out


# revision 21
# speedup vs baseline: 1.0263x; 1.0263x over previous
"""Trainium2 Bass kernel for the 81-step LSTM decoder + masked softmax.

Math (per batch row b):
    z_t = x_t @ W_x + h_{t-1} @ W_h + b          (gates i, f, g, o; 100 each)
    i,f,o = sigmoid;  g = identity
    c_t = f*c_{t-1} + i*g;  h_t = o*c_t
    out_t = softmax(where(mask_t, h_t, -inf))

Strategy: data-parallel over batch (4096 -> 8 cores x 512), h-major on device
(hidden dim on partitions, batch on free dim), identical program per core.

The x-projection runs in fp8-e4m3 DoubleRow mode (0.5 PE cycles/row, two
128-row K-tiles per instruction).  Everything is scaled by S=64 on device
(W8 = q8(S*W_x) keeps the fp8 residual of W representable) and un-scaled for
free inside the activations via ACT's scale argument:
  i,f,o gates: 1-term  z ~ x8 @ W8            (sigmoid damps fp8 noise)
  g gate:      3-term  x8@W8 + xlo@W8 + x8@Wlo (g feeds c linearly -> bf16-
               quality needed; pairs ride in the DoubleRow k-tile slots)
The recurrent matmul stays bf16 (1 cyc/row).  The LSTM elementwise chain runs
in bf16 on DVE (2x packed mode), split into two independent batch halves so
the two recurrence chains pipeline across engines.  The masked softmax:
exp(h/S) on ACT (3-step chunks), then a "masked transpose" per step -- a
plain matmul  e^T @ diag(mask01_t)  that transposes to batch-major and
applies the softmax mask in one PE pass -- then sum + normalize on the
(otherwise idle) GPSIMD engine, reciprocal on DVE.  Output is written bf16
and upconverted to fp32 on the host.
"""

import sys

if "/opt/trn_rl_repo" not in sys.path:
    sys.path.insert(0, "/opt/trn_rl_repo")

import numpy as np

P = 81       # places / timesteps
H = 100      # LSTM units
E = 512      # encoder feature width
B = 4096     # total batch
NCORES = 8
BS = B // NCORES          # 512 batch rows per core
NB = BS // 128            # 4 batch tiles of 128
NE = E // 128             # 4 feature chunks of 128
HS = BS // 2              # 256: batch half for the split recurrence chains
K = 9                     # softmax window (81 % 9 == 0)
S = 64.0                  # global gate scale (keeps fp8 W residual normal)

# gate blocks in device column order: 0=f 1=i 2=o 3=g
# reference W column order is i,f,g,o -> reorder slices
_REORD = [slice(100, 200), slice(0, 100), slice(300, 400), slice(200, 300)]

_PROGRAM = None


def _build_program():
    import concourse.bacc as bacc
    import concourse.mybir as mybir
    from concourse.tile import TileContext
    from concourse.tile_rust import add_dep_helper
    from contextlib import ExitStack

    f32 = mybir.dt.float32
    bf16 = mybir.dt.bfloat16
    fp8 = mybir.dt.float8e4
    SIG = mybir.ActivationFunctionType.Sigmoid
    EXP = mybir.ActivationFunctionType.Exp
    DR = mybir.MatmulPerfMode.DoubleRow
    ADD = mybir.AluOpType.add
    X = mybir.AxisListType.X

    nc = bacc.Bacc(None, target_bir_lowering=False)

    xin_d = nc.dram_tensor("xin", [P, 128, 4096], fp8, kind="ExternalInput")
    w8p_d = nc.dram_tensor("w8p", [128, 2, 4, 2, 112], fp8, kind="ExternalInput")
    wlp_d = nc.dram_tensor("wlp", [128, 2, 2, 112], fp8, kind="ExternalInput")
    whb_d = nc.dram_tensor("whb", [H + 1, 400], bf16, kind="ExternalInput")
    dmk_d = nc.dram_tensor("dmk", [H, P * (H + 1)], bf16, kind="ExternalInput")
    h0T_d = nc.dram_tensor("h0T", [H + 1, BS], bf16, kind="ExternalInput")
    out_d = nc.dram_tensor("out", [BS, P, H], bf16, kind="ExternalOutput")

    with ExitStack() as ctx:
        tc = ctx.enter_context(TileContext(nc))
        consts = ctx.enter_context(tc.tile_pool(name="consts", bufs=1))
        xpool = ctx.enter_context(tc.tile_pool(name="xpool", bufs=12))
        gpool = ctx.enter_context(tc.tile_pool(name="gpool", bufs=8))
        opool = ctx.enter_context(tc.tile_pool(name="opool", bufs=8))
        zpool = ctx.enter_context(tc.tile_pool(name="zpool", bufs=2, space="PSUM"))
        zgpool = ctx.enter_context(tc.tile_pool(name="zgpool", bufs=1, space="PSUM"))
        epool = ctx.enter_context(tc.tile_pool(name="epool", bufs=1, space="PSUM"))

        # prefetch the first x tiles ahead of the big consts DMAs so the PE
        # can start immediately
        xtiles = {}
        for t0 in range(4):
            xtiles[t0] = xpool.tile([128, 4096], fp8, name=f"x_{t0}", tag="x")
            nc.sync.dma_start(out=xtiles[t0], in_=xin_d[t0])

        w8p = consts.tile([128, 2, 4, 2, 112], fp8)
        nc.sync.dma_start(out=w8p, in_=w8p_d[:, :, :, :, :])
        wlp = consts.tile([128, 2, 2, 112], fp8)
        nc.sync.dma_start(out=wlp, in_=wlp_d[:, :, :, :])
        whb = consts.tile([H + 1, 400], bf16)
        nc.sync.dma_start(out=whb, in_=whb_d[:, :])
        dmk = consts.tile([H, P, H + 1], bf16)
        nc.sync.dma_start(
            out=dmk, in_=dmk_d[:, :].rearrange("p (t h) -> p t h", h=H + 1)
        )

        # h history ring: rows 0:100 = S*h_t (bf16), row 100 = 1.0 (bias rider)
        hist = consts.tile([H + 1, K, BS], bf16)
        nc.sync.dma_start(out=hist[:, K - 1, :], in_=h0T_d[:, :])
        for j in range(K - 1):
            nc.sync.dma_start(out=hist[H : H + 1, j, :], in_=h0T_d[H : H + 1, :])
        cC = consts.tile([H, BS], bf16)       # S*c_t, persistent
        nc.vector.memset(cC, 0.0)
        ew = consts.tile([H, K, BS], bf16)    # exp(h/S) window staging

        # ACT LUT ops chained in program order (avoid sigmoid<->exp table churn)
        act_prev = [None]

        def act_ordered(bi):
            if act_prev[0] is not None:
                add_dep_helper(bi.ins, act_prev[0].ins, sync=False, reason="act order")
            act_prev[0] = bi

        def softmax_tail(tau):
            slot = tau % K
            # masked transpose + fused masked sum:
            #   eT[b, h] = sum_hh e[hh, b] * D[hh, h]   (D = diag(mask01_t))
            #   eT[b, 100] = sum_hh e[hh, b] * mask01_t[hh]   (softmax denom)
            eT = epool.tile([128, NB, H + 1], f32, name=f"eT_{tau}", tag="eT")
            for k in range(NB):
                nc.tensor.matmul(
                    eT[:, k, :],
                    ew[:, slot, 128 * k : 128 * (k + 1)],
                    dmk[:, tau, :],
                    start=True,
                    stop=True,
                )
            r = opool.tile([128, NB], f32, name=f"r_{tau}", tag="r")
            nc.vector.reciprocal(r, eT[:, :, H])
            ostg = opool.tile([128, NB, H], bf16, name=f"o_{tau}", tag="ostg")
            nc.vector.tensor_mul(
                ostg, eT[:, :, 0:H], r[:, :, None].broadcast_to([128, NB, H])
            )
            nc.sync.dma_start(
                out=out_d.rearrange("(k p) t h -> p k t h", p=128)[:, :, tau, :],
                in_=ostg,
            )

        for t in range(P):
            if t in xtiles:
                xt = xtiles[t]
            else:
                xt = xpool.tile([128, 4096], fp8, name=f"x_{t}", tag="x")
                nc.sync.dma_start(out=xt, in_=xin_d[t])

            # contiguous [2, 512] DR moving pair at (kind, ep)
            def xpair(kind, ep):
                off = kind * 2048 + ep * 1024
                return xt[:, off : off + 1024].rearrange("p (j n) -> p j n", j=2)

            prev = (t - 1) % K
            zf = zpool.tile([H, 3, BS], f32, name=f"zf_{t}", tag="zf")
            zg = zgpool.tile([H, BS], f32, name=f"zg_{t}", tag="zg")

            # ---- x-projection (no h dependency) ----
            # DoubleRow start=True zeroes the whole PSUM bank, so exactly one
            # start per bank per step: the bank's first matmul. All regions
            # accumulate per-address after that (skip_group_check).
            for blk in range(3):               # f, i, o : 1-term fp8 DR
                for ep in range(2):
                    nc.tensor.matmul(
                        zf[:, blk, :],
                        w8p[:, ep, blk, :, 0:100],
                        xpair(0, ep),
                        start=(ep == 0),
                        stop=False,
                        perf_mode=DR,
                        skip_group_check=True,
                    )
            # g : 3-term fp8 DR (x8@W8g + xlo@W8g + x8@Wlog)
            for ti, (kind, wt) in enumerate(((0, w8p[:, :, 3]), (1, w8p[:, :, 3]), (0, wlp))):
                for ep in range(2):
                    nc.tensor.matmul(
                        zg[:, :],
                        wt[:, ep, :, 0:100],
                        xpair(kind, ep),
                        start=(ti == 0 and ep == 0),
                        stop=False,
                        perf_mode=DR,
                        skip_group_check=True,
                    )

            # ---- recurrent part + gate chain, per batch half ----
            for hf in range(2):
                bs_ = slice(HS * hf, HS * (hf + 1))
                for blk in range(3):
                    nc.tensor.matmul(
                        zf[:, blk, bs_],
                        whb[:, 100 * blk : 100 * (blk + 1)],
                        hist[:, prev, bs_],
                        start=False,
                        stop=(hf == 1),
                        skip_group_check=True,
                    )
                nc.tensor.matmul(
                    zg[:, bs_],
                    whb[:, 300:400],
                    hist[:, prev, bs_],
                    start=False,
                    stop=(hf == 1),
                    skip_group_check=True,
                )

                fio = gpool.tile([H, 3, BS], bf16, name=f"fio_{t}_{hf}", tag="fio")
                act_ordered(
                    nc.scalar.activation(
                        fio[:, :, bs_], zf[:, :, bs_], SIG, scale=1.0 / S
                    )
                )
                t1 = gpool.tile([H, BS], bf16, name=f"t1_{t}_{hf}", tag="t1")
                nc.gpsimd.tensor_mul(t1[:, bs_], fio[:, 0, bs_], cC[:, bs_])
                t2 = gpool.tile([H, BS], bf16, name=f"t2_{t}_{hf}", tag="t2")
                nc.vector.tensor_mul(t2[:, bs_], fio[:, 1, bs_], zg[:, bs_])
                nc.vector.tensor_add(cC[:, bs_], t1[:, bs_], t2[:, bs_])
                nc.vector.tensor_mul(
                    hist[0:H, t % K, bs_], fio[:, 2, bs_], cC[:, bs_]
                )

            if t % K == K - 1:
                for c0 in range(0, K, 3):
                    act_ordered(
                        nc.scalar.activation(
                            ew[:, c0 : c0 + 3, :],
                            hist[0:H, c0 : c0 + 3, :],
                            EXP,
                            scale=1.0 / S,
                        )
                    )
                for tau in range(t - K + 1, t + 1):
                    softmax_tail(tau)

    nc.compile()
    return nc


def _get_program():
    global _PROGRAM
    if _PROGRAM is None:
        _PROGRAM = _build_program()
    return _PROGRAM


def _prep_in_maps(h_enc, h0, W_x, W_h, b, mask):
    import ml_dtypes

    e4 = ml_dtypes.float8_e4m3
    bf16 = ml_dtypes.bfloat16
    h_enc = np.asarray(h_enc, dtype=np.float32)
    h0 = np.asarray(h0, dtype=np.float32)
    W_x = np.asarray(W_x, dtype=np.float32)
    W_h = np.asarray(W_h, dtype=np.float32)
    b = np.asarray(b, dtype=np.float32)
    mask = np.asarray(mask)

    def q8(x):
        return np.clip(x, -240.0, 240.0).astype(e4)

    # reorder W columns to (f, i, o, g)
    Wp = np.concatenate([W_x[:, s] for s in _REORD], axis=1)          # [512, 400]
    bp = np.concatenate([b[s] for s in _REORD])                       # [400]
    Whp = np.concatenate([W_h[:, s] for s in _REORD], axis=1)         # [100, 400]

    W8 = q8(S * Wp)
    Wlo = q8(S * Wp - W8.astype(np.float32))
    W8c = W8.reshape(NE, 128, 400)
    Wloc = Wlo.reshape(NE, 128, 400)
    # W8 pairs for all 4 gate blocks: [128, ep, blk, j, 112], (ep,j) -> e-chunk
    # 2*ep+j (pair stride padded to 112: DoubleRow ldweights needs stride%16==0)
    w8p = np.zeros((128, 2, 4, 2, 112), e4)
    w8p[:, :, :, :, 0:100] = (
        W8c.reshape(2, 2, 128, 4, 100)     # [ep, j, p, blk, 100]
        .transpose(2, 0, 3, 1, 4)          # [p, ep, blk, j, 100]
    )
    # g term3: Wlo pairs: [128, ep, j, 112]
    wlp = np.zeros((128, 2, 2, 112), e4)
    wlp[:, :, :, 0:100] = (
        Wloc[:, :, 300:400].reshape(2, 2, 128, 100).transpose(2, 0, 1, 3)
    )

    whb = np.concatenate([Whp, (S * bp)[None, :]], axis=0).astype(bf16)  # [101, 400]

    # masked-transpose blocks: dmk[hh, t, h<100] = (hh==h)*mask01[t,h];
    # column 100 = mask01[t, hh] (produces the masked softmax sum in-pass)
    m01 = mask.astype(np.float32)                                     # [81, 100]
    dmk = np.zeros((H, P, H + 1), np.float32)
    ii = np.arange(H)
    dmk[ii, :, ii] = m01.T[ii]                                        # diag per t
    dmk[:, :, H] = m01.T                                              # sum column
    dmk = dmk.reshape(H, P * (H + 1)).astype(bf16)

    in_maps = []
    xT = np.empty((P, E, BS), np.float32)
    for c in range(NCORES):
        shard = h_enc[c * BS : (c + 1) * BS]                          # [BS, P, E]
        for t in range(P):
            xT[t] = shard[:, t, :].T
        x8 = q8(xT)
        xlo = q8(xT - x8.astype(np.float32))
        # xin [P, 128, kind, ep, j, 512]: each DR rhs a contiguous [2,512]
        xin = np.empty((P, 128, 2, 2, 2, 512), e4)
        for kind, xq in ((0, x8), (1, xlo)):
            a = xq.reshape(P, 2, 2, 128, 512)           # [t, ep, j, p, n]
            xin[:, :, kind] = a.transpose(0, 3, 1, 2, 4)
        xin = xin.reshape(P, 128, 4096)
        h0T = np.concatenate(
            [S * h0[c * BS : (c + 1) * BS].T, np.ones((1, BS), np.float32)], axis=0
        ).astype(bf16)
        in_maps.append(
            {
                "xin": xin,
                "w8p": w8p,
                "wlp": wlp,
                "whb": whb,
                "dmk": dmk,
                "h0T": np.ascontiguousarray(h0T),
            }
        )
    return in_maps


def run(inputs: dict, trace: bool = False):
    """Run on 8 cores; returns (full_output, exec_time_ns_or_None)."""
    from concourse.bass_utils import run_bass_kernel_spmd

    nc = _get_program()
    in_maps = _prep_in_maps(**inputs)
    res = run_bass_kernel_spmd(
        nc, in_maps, core_ids=list(range(NCORES)), trace=trace
    )
    out = np.concatenate(
        [r["out"].astype(np.float32) for r in res.results], axis=0
    )
    return out, res.exec_time_ns


def kernel(**inputs) -> np.ndarray:
    out, _ = run(inputs, trace=False)
    return out
